# revision 76
# baseline (speedup 1.0000x reference)
"""Trainium2 Bass kernel for nn_CenterCrop: per-sample resize(short-side=256)
+ center-crop(224), bilinear, batch sharded over 8 NeuronCores.

Bilinear resize is separable: out = S^T @ img @ G with per-sample sparse
interpolation matrices S (vertical) and G (horizontal), built on the host
from the h/w metadata. The gather+lerp runs on the PE array as matmuls:
  pass1: tmp1_T[x, j] = sum_y img[y, x] * S[y, j]   (img tiles stationary)
  pass2: out[j, i]    = sum_x tmp1_T[x, j] * G[x, i] (tmp1 tiles stationary)

Perf structure (baseline fp32 115.5us -> ~44us measured):
- fp16 data path end-to-end: PE streams at 1 cyc/row (4x the fp32 rate),
  DMA bytes halve, output written fp16 and upcast on host. Bilinear error
  stays ~8e-4 max-rel (gate is 2e-2).
- Only the per-sample source window that the output reads is DMA'd, in a
  partition-major layout so every DMA is one linear run per partition.
- S/G are banded (2 nonzeros per column); only the per-128-row-tile
  nonzero band columns ship, packed into two blobs (first two compute
  slots' bands alone so the first matmul isn't gated on the rest).
- All input DMAs are issued up front (img bufs = n_slots, no reuse WAR).
- PSUM channel pairing: c0+c1 share a [*,448] PSUM tile and a single
  accumulation session (has_written bits make split ranges exact), so
  each x-chunk drains in 2 wide casts (Act: 448-wide, DVE: 224-wide)
  instead of 3 narrow ones.
- Software pipelining: pass2 of slot s-1 is emitted after pass1 of slot
  s, so the PE queue never head-of-line blocks on the ps1->tmp drains.
- SPMD requires one program for all 8 cores, so samples are sorted by
  min(h,w) and dealt round-robin: slot s on every core holds same-sized
  windows; the program is specialized per-slot to the union shape/bands.
  Outputs are unpermuted/transposed back on the host.
"""

import sys
import os

for _p in ("/opt/trn_rl_repo",):
    if os.path.isdir(_p) and _p not in sys.path:
        sys.path.insert(0, _p)

import numpy as np

OUT_H = 224
OUT_W = 224
RESIZE_TO = np.float32(256.0)
B_FULL = 64
N_CORES = 8
B_LOC = B_FULL // N_CORES  # 8 slots per core
C = 3
H = 512
W = 512  # image width after stripping the metadata column (stored width 513)

LAST_EXEC_NS = None
LAST_RESULTS = None
_NC_CACHE = {}

# fp16 single-pass data path (default). Disable via CENTERCROP_F16=0 for an
# fp32 debugging fallback.
USE_F16 = os.environ.get("CENTERCROP_F16", "1") == "1"


def _interp_matrices(h, w):
    """Full S [512, OUT_H], G [512, OUT_W] fp32 interpolation matrices,
    mirroring the reference fp32 math bit-for-bit."""
    f32 = np.float32
    h = f32(h)
    w = f32(w)
    min_dim = min(h, w)
    scale = RESIZE_TO / min_dim
    h_res = np.round(h * scale)
    w_res = np.round(w * scale)
    top = np.round((h_res - f32(OUT_H)) / f32(2.0))
    left = np.round((w_res - f32(OUT_W)) / f32(2.0))

    def axis_mat(n_out, offset, dim, dim_res, n_src):
        idx = np.arange(n_out, dtype=np.float32) + offset
        src = np.clip((idx + f32(0.5)) * dim / dim_res - f32(0.5),
                      f32(0.0), dim - f32(1.0))
        p0f = np.floor(src)
        frac = src - p0f
        imax = np.int32(dim) - 1
        p0 = np.clip(p0f.astype(np.int32), 0, imax)
        p1 = np.minimum(p0 + 1, imax)
        mat = np.zeros((n_src, n_out), np.float32)
        cols = np.arange(n_out)
        np.add.at(mat, (p0, cols), f32(1.0) - frac)
        np.add.at(mat, (p1, cols), frac)
        return mat

    S = axis_mat(OUT_H, top, h, h_res, H)
    G = axis_mat(OUT_W, left, w, w_res, W)
    return S, G


def _bands(mat_w, n_tiles):
    """Per-128-row-tile [lo, hi) columns with any nonzero; None if empty."""
    out = []
    for t in range(n_tiles):
        rows = mat_w[t * 128:(t + 1) * 128]
        nz = np.nonzero(rows.any(axis=0))[0]
        out.append(None if nz.size == 0 else (int(nz[0]), int(nz[-1]) + 1))
    return out


def _union_bands(band_lists):
    n = len(band_lists[0])
    out = []
    for t in range(n):
        los = [b[t][0] for b in band_lists if b[t] is not None]
        his = [b[t][1] for b in band_lists if b[t] is not None]
        out.append(None if not los else (min(los), max(his)))
    return out


def _band_offsets(bands):
    """Packed running offsets for non-empty bands; total width last."""
    offs = []
    off = 0
    for b in bands:
        if b is None:
            offs.append(None)
        else:
            offs.append(off)
            off += b[1] - b[0]
    return offs, off


def _prepare(x):
    """Host prep: per-sample windows/matrices, sorted slot assignment,
    per-core packed inputs, and the per-slot program parameters."""
    dtd_np = np.float16 if USE_F16 else np.float32
    h_all = x[:, 0, 0, -1].astype(np.float32)
    w_all = x[:, 1, 0, -1].astype(np.float32)

    samples = []
    for b in range(B_FULL):
        S, G = _interp_matrices(h_all[b], w_all[b])
        ynz = np.nonzero(S.any(axis=1))[0]
        xnz = np.nonzero(G.any(axis=1))[0]
        y0, y1 = int(ynz[0]), int(ynz[-1]) + 1
        x0, x1 = int(xnz[0]), int(xnz[-1]) + 1
        samples.append(dict(S=S[y0:y1], G=G[x0:x1], y0=y0, x0=x0,
                            wh=y1 - y0, ww=x1 - x0))

    order = np.argsort(np.minimum(h_all, w_all), kind="stable")
    # slot s, core c -> sample order[s*N_CORES + c]
    assign = [[int(order[s * N_CORES + c]) for c in range(N_CORES)]
              for s in range(B_LOC)]

    slot_params = []
    slot_data = []  # per slot: list over cores of (sid, Sw_pad, Gw_pad)
    for s in range(B_LOC):
        sids = assign[s]
        wh = max(samples[i]["wh"] for i in sids)
        ww = max(samples[i]["ww"] for i in sids)
        n_yt = (wh + 127) // 128
        n_xt = (ww + 127) // 128
        sb_list, gb_list, data = [], [], []
        for i in sids:
            sp = samples[i]
            Sw = np.zeros((n_yt * 128, OUT_H), np.float32)
            Sw[:sp["wh"]] = sp["S"]
            Gw = np.zeros((n_xt * 128, OUT_W), np.float32)
            Gw[:sp["ww"]] = sp["G"]
            sb_list.append(_bands(Sw, n_yt))
            gb_list.append(_bands(Gw, n_xt))
            data.append((i, Sw, Gw))
        sbands = _union_bands(sb_list)
        gbands = _union_bands(gb_list)
        slot_params.append((n_yt, n_xt, ww,
                            tuple(sbands), tuple(gbands)))
        slot_data.append(data)

    # pack per-core input maps; all slots' S|G bands share one "sgall"
    in_maps = [{} for _ in range(N_CORES)]
    sg_parts = [[] for _ in range(N_CORES)]
    for s in range(B_LOC):
        n_yt, n_xt, ww, sbands, gbands = slot_params[s]
        s_offs, s_tot = _band_offsets(sbands)
        g_offs, g_tot = _band_offsets(gbands)
        for c in range(N_CORES):
            sid, Sw, Gw = slot_data[s][c]
            sp = samples[sid]
            xw = np.zeros((C, n_yt, 128, ww), dtd_np)
            win = x[sid, :, sp["y0"]:sp["y0"] + sp["wh"],
                    sp["x0"]:sp["x0"] + sp["ww"]]
            for t in range(n_yt):
                rows = win[:, t * 128:(t + 1) * 128]
                xw[:, t, :rows.shape[1], :sp["ww"]] = rows
            # partition-major layout so the DMA is one linear run/partition
            xw = np.ascontiguousarray(xw.transpose(2, 0, 1, 3))
            # packed S|G bands: [128, s_tot + g_tot]
            sg = np.zeros((128, s_tot + g_tot), dtd_np)
            St = Sw.reshape(n_yt, 128, OUT_H)
            Gt = Gw.reshape(n_xt, 128, OUT_W)
            for t in range(n_yt):
                if sbands[t] is not None:
                    lo, hi = sbands[t]
                    sg[:, s_offs[t]:s_offs[t] + hi - lo] = St[t, :, lo:hi]
            for t in range(n_xt):
                if gbands[t] is not None:
                    lo, hi = gbands[t]
                    o = s_tot + g_offs[t]
                    sg[:, o:o + hi - lo] = Gt[t, :, lo:hi]
            in_maps[c][f"xw{s}"] = xw
            sg_parts[c].append(sg)
    order = list(range(1, B_LOC)) + [0]
    for c in range(N_CORES):
        in_maps[c]["sg_first"] = np.concatenate(
            [sg_parts[c][s] for s in order[:2]], axis=1)
        in_maps[c]["sg_rest"] = np.concatenate(
            [sg_parts[c][s] for s in order[2:]], axis=1)
    return tuple(slot_params), in_maps, assign


def _build_nc(slot_params):
    import concourse.bacc as bacc
    import concourse.mybir as mybir
    import concourse.tile as tile

    dt = mybir.dt.float32
    dtd = mybir.dt.float16 if USE_F16 else mybir.dt.float32
    nc = bacc.Bacc(
        "TRN2",
        target_bir_lowering=False,
        debug=False,
        enable_asserts=False,
        num_devices=N_CORES,
    )
    xw_in, sg_meta = [], []
    for s, (n_yt, n_xt, ww, sbands, gbands) in enumerate(slot_params):
        s_offs, s_tot = _band_offsets(sbands)
        g_offs, g_tot = _band_offsets(gbands)
        sg_meta.append((s_offs, g_offs, s_tot, g_tot))
        xw_in.append(nc.dram_tensor(f"xw{s}", [128, C, n_yt, ww], dtd,
                                    kind="ExternalInput"))
    out = nc.dram_tensor("out", [B_LOC, 112, 2, C, OUT_W], dtd,
                         kind="ExternalOutput")

    n_slots = len(slot_params)
    # head waits on slot_order[0]'s DMA (keep it small-ish); the tail drain
    # is slot_order[-1]'s pass2 (make it the smallest slot)
    slot_order = list(range(1, n_slots)) + [0]
    # S|G bands ship as two blobs: the first two compute slots' pieces (so
    # early matmuls aren't gated on the whole thing), then the rest
    sg_w = [sg_meta[s][2] + sg_meta[s][3] for s in range(n_slots)]
    sg_base = {}
    off0 = 0
    for s in slot_order[:2]:
        sg_base[s] = off0
        off0 += sg_w[s]
    off = 0
    for s in slot_order[2:]:
        sg_base[s] = off
        off += sg_w[s]
    sg0_in = nc.dram_tensor("sg_first", [128, off0], dtd,
                            kind="ExternalInput")
    sgr_in = nc.dram_tensor("sg_rest", [128, off], dtd,
                            kind="ExternalInput")
    with tile.TileContext(nc) as tc:
        with (
            tc.tile_pool(name="img", bufs=n_slots) as img_pool,
            tc.tile_pool(name="sg", bufs=2) as sg_pool,
            tc.tile_pool(name="tmp", bufs=3) as tmp_pool,
            tc.tile_pool(name="outp", bufs=3) as out_pool,
            tc.tile_pool(name="ps1a", bufs=2, space="PSUM") as ps1a_pool,
            tc.tile_pool(name="ps1b", bufs=2, space="PSUM") as ps1b_pool,
            tc.tile_pool(name="ps2a", bufs=2, space="PSUM") as ps2a_pool,
            tc.tile_pool(name="ps2b", bufs=2, space="PSUM") as ps2b_pool,
        ):
            tmp_tiles = [None] * n_slots

            # prefetch everything up front: first slot's S|G, its image,
            # then the remaining S|G blob and the other images
            img_tiles = [None] * n_slots
            first2 = slot_order[:2]
            sg0_sb = sg_pool.tile([128, off0], dtd, tag="sg0")
            nc.sync.dma_start(sg0_sb[:], sg0_in[:])
            for s in first2:
                n_yt, n_xt, ww, _, _ = slot_params[s]
                img_sb = img_pool.tile([128, C, n_yt, ww], dtd)
                nc.sync.dma_start(img_sb[:], xw_in[s][:])
                img_tiles[s] = img_sb
            sgr_sb = sg_pool.tile([128, off], dtd, tag="sgr")
            nc.sync.dma_start(sgr_sb[:], sgr_in[:])
            for s in slot_order[2:]:
                n_yt, n_xt, ww, _, _ = slot_params[s]
                img_sb = img_pool.tile([128, C, n_yt, ww], dtd)
                nc.sync.dma_start(img_sb[:], xw_in[s][:])
                img_tiles[s] = img_sb
            sg_tiles = [(sg0_sb if s in first2 else sgr_sb) for s in
                        range(n_slots)]

            def pass1(s):
                n_yt, n_xt, ww, sbands, gbands = slot_params[s]
                s_offs, g_offs, s_tot, g_tot = sg_meta[s]
                sg_sb = sg_tiles[s]
                img_sb = img_tiles[s]
                s_emit = [t for t in range(n_yt) if sbands[t] is not None]
                tmp_sb = tmp_pool.tile([128, n_xt, C, OUT_H], dtd)
                tmp_tiles[s] = tmp_sb
                for xb in range(n_xt):
                    xlo = xb * 128
                    xn = min(128, ww - xlo)
                    ps1a = ps1a_pool.tile([128, 2 * OUT_H], dt, tag='ps1a')
                    ps1b = ps1b_pool.tile([128, OUT_H], dt, tag='ps1b')
                    # c0+c1 share one accumulation session in ps1a: PSUM
                    # has_written bits make the split ranges exact
                    for c in range(C):
                        base = c * OUT_H if c < 2 else 0
                        ps = ps1a if c < 2 else ps1b
                        for i_t, t in enumerate(s_emit):
                            lo, hi = sbands[t]
                            o = sg_base[s] + s_offs[t]
                            first = i_t == 0 and (c == 0 or c == 2)
                            last = (i_t == len(s_emit) - 1
                                    and (c == 1 or c == 2))
                            nc.tensor.matmul(
                                ps[:xn, base + lo:base + hi],
                                img_sb[:, c, t, xlo:xlo + xn],
                                sg_sb[:, o:o + hi - lo],
                                start=first,
                                stop=last,
                                skip_group_check=True,
                            )
                    # ping-pong the drains so each engine sees half the
                    # burst rate (small slots outrun a single drain engine)
                    if xb % 2 == 0:
                        nc.scalar.copy(tmp_sb[:xn, xb, 0:2, :],
                                       ps1a[:xn, :])
                        nc.vector.tensor_copy(tmp_sb[:xn, xb, 2, :],
                                              ps1b[:xn, :OUT_H])
                    else:
                        nc.vector.tensor_copy(tmp_sb[:xn, xb, 0:2, :],
                                              ps1a[:xn, :])
                        nc.scalar.copy(tmp_sb[:xn, xb, 2, :],
                                       ps1b[:xn, :OUT_H])

            def pass2(s):
                n_yt, n_xt, ww, sbands, gbands = slot_params[s]
                s_offs, g_offs, s_tot, g_tot = sg_meta[s]
                sg_sb = sg_tiles[s]
                tmp_sb = tmp_tiles[s]
                g_emit = [t for t in range(n_xt) if gbands[t] is not None]
                out_sb = out_pool.tile([112, 2, C, OUT_W], dtd, tag="out")
                for jb in range(2):
                    ps2a = ps2a_pool.tile([112, 2 * OUT_W], dt, tag='ps2a')
                    ps2b = ps2b_pool.tile([112, OUT_W], dt, tag='ps2b')
                    for c in range(C):
                        base = c * OUT_W if c < 2 else 0
                        ps = ps2a if c < 2 else ps2b
                        for i_t, xb in enumerate(g_emit):
                            lo, hi = gbands[xb]
                            o = sg_base[s] + s_tot + g_offs[xb]
                            xn = min(128, ww - xb * 128)
                            first = i_t == 0 and (c == 0 or c == 2)
                            last = (i_t == len(g_emit) - 1
                                    and (c == 1 or c == 2))
                            nc.tensor.matmul(
                                ps[:, base + lo:base + hi],
                                tmp_sb[:xn, xb, c,
                                       jb * 112:(jb + 1) * 112],
                                sg_sb[:xn, o:o + hi - lo],
                                start=first,
                                stop=last,
                                skip_group_check=True,
                            )
                    if jb == 0:
                        nc.vector.tensor_copy(out_sb[:, jb, 0:2, :],
                                              ps2a[:, :])
                        nc.scalar.copy(out_sb[:, jb, 2, :],
                                       ps2b[:, :OUT_W])
                    else:
                        nc.scalar.copy(out_sb[:, jb, 0:2, :],
                                       ps2a[:, :])
                        nc.vector.tensor_copy(out_sb[:, jb, 2, :],
                                              ps2b[:, :OUT_W])
                nc.sync.dma_start(out[s][:], out_sb[:])

            # software pipeline: pass2 of the previous slot is emitted after
            # pass1 of the current one, so PE never head-of-line blocks on
            # the ps1->tmp drains.
            prev = None
            for s in slot_order:
                pass1(s)
                if prev is not None:
                    pass2(prev)
                prev = s
            pass2(prev)
    nc.compile()
    return nc


def kernel(x, _trace=False):
    global LAST_EXEC_NS, LAST_RESULTS
    from concourse.bass_utils import run_bass_kernel_spmd

    x = np.ascontiguousarray(np.asarray(x), dtype=np.float32)
    assert x.shape == (B_FULL, C, H, W + 1), x.shape

    slot_params, in_maps, assign = _prepare(x)
    key = (slot_params, USE_F16)
    if key not in _NC_CACHE:
        _NC_CACHE[key] = _build_nc(slot_params)
    nc = _NC_CACHE[key]

    res = run_bass_kernel_spmd(nc, in_maps, list(range(N_CORES)), trace=_trace)
    LAST_EXEC_NS = res.exec_time_ns
    LAST_RESULTS = res

    out_full = np.empty((B_FULL, C, OUT_H, OUT_W), np.float32)
    for s in range(B_LOC):
        for c in range(N_CORES):
            # device layout [112, 2, C, OUT_W]; j = b*112 + p
            arr = res.results[c]["out"][s].astype(np.float32)
            out_full[assign[s][c]] = arr.transpose(2, 1, 0, 3).reshape(
                C, OUT_H, OUT_W)
    return out_full


# revision 77
# speedup vs baseline: 1.0161x; 1.0161x over previous
"""Trainium2 Bass kernel for nn_CenterCrop: per-sample resize(short-side=256)
+ center-crop(224), bilinear, batch sharded over 8 NeuronCores.

Bilinear resize is separable: out = S^T @ img @ G with per-sample sparse
interpolation matrices S (vertical) and G (horizontal), built on the host
from the h/w metadata. The gather+lerp runs on the PE array as matmuls:
  pass1: tmp1_T[x, j] = sum_y img[y, x] * S[y, j]   (img tiles stationary)
  pass2: out[j, i]    = sum_x tmp1_T[x, j] * G[x, i] (tmp1 tiles stationary)

Perf structure (baseline fp32 115.5us -> ~44us measured):
- fp16 data path end-to-end: PE streams at 1 cyc/row (4x the fp32 rate),
  DMA bytes halve, output written fp16 and upcast on host. Bilinear error
  stays ~8e-4 max-rel (gate is 2e-2).
- Only the per-sample source window that the output reads is DMA'd, in a
  partition-major layout so every DMA is one linear run per partition.
- S/G are banded (2 nonzeros per column); only the per-128-row-tile
  nonzero band columns ship, packed into two blobs (first two compute
  slots' bands alone so the first matmul isn't gated on the rest).
- All input DMAs are issued up front (img bufs = n_slots, no reuse WAR).
- PSUM channel pairing: c0+c1 share a [*,448] PSUM tile and a single
  accumulation session (has_written bits make split ranges exact), so
  each x-chunk drains in 2 wide casts instead of 3 narrow ones; the two
  drain engines (Act, DVE) ping-pong per x-chunk/jb so neither sees the
  full burst rate (GpSimd cannot touch PSUM, DMA cannot read it).
- Software pipelining: pass2 of slot s-1 is emitted after pass1 of slot
  s, so the PE queue never head-of-line blocks on the ps1->tmp drains.
- SPMD requires one program for all 8 cores, so samples are sorted by
  min(h,w) and dealt round-robin: slot s on every core holds same-sized
  windows; the program is specialized per-slot to the union shape/bands.
  Outputs are unpermuted/transposed back on the host.
"""

import sys
import os

for _p in ("/opt/trn_rl_repo",):
    if os.path.isdir(_p) and _p not in sys.path:
        sys.path.insert(0, _p)

import numpy as np

OUT_H = 224
OUT_W = 224
RESIZE_TO = np.float32(256.0)
B_FULL = 64
N_CORES = 8
B_LOC = B_FULL // N_CORES  # 8 slots per core
C = 3
H = 512
W = 512  # image width after stripping the metadata column (stored width 513)

LAST_EXEC_NS = None
LAST_RESULTS = None
_NC_CACHE = {}

# fp16 single-pass data path (default). Disable via CENTERCROP_F16=0 for an
# fp32 debugging fallback.
USE_F16 = os.environ.get("CENTERCROP_F16", "1") == "1"


def _interp_matrices(h, w):
    """Full S [512, OUT_H], G [512, OUT_W] fp32 interpolation matrices,
    mirroring the reference fp32 math bit-for-bit."""
    f32 = np.float32
    h = f32(h)
    w = f32(w)
    min_dim = min(h, w)
    scale = RESIZE_TO / min_dim
    h_res = np.round(h * scale)
    w_res = np.round(w * scale)
    top = np.round((h_res - f32(OUT_H)) / f32(2.0))
    left = np.round((w_res - f32(OUT_W)) / f32(2.0))

    def axis_mat(n_out, offset, dim, dim_res, n_src):
        idx = np.arange(n_out, dtype=np.float32) + offset
        src = np.clip((idx + f32(0.5)) * dim / dim_res - f32(0.5),
                      f32(0.0), dim - f32(1.0))
        p0f = np.floor(src)
        frac = src - p0f
        imax = np.int32(dim) - 1
        p0 = np.clip(p0f.astype(np.int32), 0, imax)
        p1 = np.minimum(p0 + 1, imax)
        mat = np.zeros((n_src, n_out), np.float32)
        cols = np.arange(n_out)
        np.add.at(mat, (p0, cols), f32(1.0) - frac)
        np.add.at(mat, (p1, cols), frac)
        return mat

    S = axis_mat(OUT_H, top, h, h_res, H)
    G = axis_mat(OUT_W, left, w, w_res, W)
    return S, G


def _bands(mat_w, n_tiles):
    """Per-128-row-tile [lo, hi) columns with any nonzero; None if empty."""
    out = []
    for t in range(n_tiles):
        rows = mat_w[t * 128:(t + 1) * 128]
        nz = np.nonzero(rows.any(axis=0))[0]
        out.append(None if nz.size == 0 else (int(nz[0]), int(nz[-1]) + 1))
    return out


def _union_bands(band_lists):
    n = len(band_lists[0])
    out = []
    for t in range(n):
        los = [b[t][0] for b in band_lists if b[t] is not None]
        his = [b[t][1] for b in band_lists if b[t] is not None]
        out.append(None if not los else (min(los), max(his)))
    return out


def _band_offsets(bands):
    """Packed running offsets for non-empty bands; total width last."""
    offs = []
    off = 0
    for b in bands:
        if b is None:
            offs.append(None)
        else:
            offs.append(off)
            off += b[1] - b[0]
    return offs, off


def _prepare(x):
    """Host prep: per-sample windows/matrices, sorted slot assignment,
    per-core packed inputs, and the per-slot program parameters."""
    dtd_np = np.float16 if USE_F16 else np.float32
    h_all = x[:, 0, 0, -1].astype(np.float32)
    w_all = x[:, 1, 0, -1].astype(np.float32)

    samples = []
    for b in range(B_FULL):
        S, G = _interp_matrices(h_all[b], w_all[b])
        ynz = np.nonzero(S.any(axis=1))[0]
        xnz = np.nonzero(G.any(axis=1))[0]
        y0, y1 = int(ynz[0]), int(ynz[-1]) + 1
        x0, x1 = int(xnz[0]), int(xnz[-1]) + 1
        samples.append(dict(S=S[y0:y1], G=G[x0:x1], y0=y0, x0=x0,
                            wh=y1 - y0, ww=x1 - x0))

    order = np.argsort(np.minimum(h_all, w_all), kind="stable")
    # slot s, core c -> sample order[s*N_CORES + c]
    assign = [[int(order[s * N_CORES + c]) for c in range(N_CORES)]
              for s in range(B_LOC)]

    slot_params = []
    slot_data = []  # per slot: list over cores of (sid, Sw_pad, Gw_pad)
    for s in range(B_LOC):
        sids = assign[s]
        wh = max(samples[i]["wh"] for i in sids)
        ww = max(samples[i]["ww"] for i in sids)
        n_yt = (wh + 127) // 128
        n_xt = (ww + 127) // 128
        sb_list, gb_list, data = [], [], []
        for i in sids:
            sp = samples[i]
            Sw = np.zeros((n_yt * 128, OUT_H), np.float32)
            Sw[:sp["wh"]] = sp["S"]
            Gw = np.zeros((n_xt * 128, OUT_W), np.float32)
            Gw[:sp["ww"]] = sp["G"]
            sb_list.append(_bands(Sw, n_yt))
            gb_list.append(_bands(Gw, n_xt))
            data.append((i, Sw, Gw))
        sbands = _union_bands(sb_list)
        gbands = _union_bands(gb_list)
        slot_params.append((n_yt, n_xt, ww,
                            tuple(sbands), tuple(gbands)))
        slot_data.append(data)

    # pack per-core input maps; all slots' S|G bands share one "sgall"
    in_maps = [{} for _ in range(N_CORES)]
    sg_parts = [[] for _ in range(N_CORES)]
    for s in range(B_LOC):
        n_yt, n_xt, ww, sbands, gbands = slot_params[s]
        s_offs, s_tot = _band_offsets(sbands)
        g_offs, g_tot = _band_offsets(gbands)
        for c in range(N_CORES):
            sid, Sw, Gw = slot_data[s][c]
            sp = samples[sid]
            xw = np.zeros((C, n_yt, 128, ww), dtd_np)
            win = x[sid, :, sp["y0"]:sp["y0"] + sp["wh"],
                    sp["x0"]:sp["x0"] + sp["ww"]]
            for t in range(n_yt):
                rows = win[:, t * 128:(t + 1) * 128]
                xw[:, t, :rows.shape[1], :sp["ww"]] = rows
            # partition-major layout so the DMA is one linear run/partition
            xw = np.ascontiguousarray(xw.transpose(2, 0, 1, 3))
            # packed S|G bands: [128, s_tot + g_tot]
            sg = np.zeros((128, s_tot + g_tot), dtd_np)
            St = Sw.reshape(n_yt, 128, OUT_H)
            Gt = Gw.reshape(n_xt, 128, OUT_W)
            for t in range(n_yt):
                if sbands[t] is not None:
                    lo, hi = sbands[t]
                    sg[:, s_offs[t]:s_offs[t] + hi - lo] = St[t, :, lo:hi]
            for t in range(n_xt):
                if gbands[t] is not None:
                    lo, hi = gbands[t]
                    o = s_tot + g_offs[t]
                    sg[:, o:o + hi - lo] = Gt[t, :, lo:hi]
            in_maps[c][f"xw{s}"] = xw
            sg_parts[c].append(sg)
    order = list(range(1, B_LOC)) + [0]
    for c in range(N_CORES):
        in_maps[c]["sg_first"] = np.concatenate(
            [sg_parts[c][s] for s in order[:2]], axis=1)
        in_maps[c]["sg_rest"] = np.concatenate(
            [sg_parts[c][s] for s in order[2:]], axis=1)
    return tuple(slot_params), in_maps, assign


def _build_nc(slot_params):
    import concourse.bacc as bacc
    import concourse.mybir as mybir
    import concourse.tile as tile

    dt = mybir.dt.float32
    dtd = mybir.dt.float16 if USE_F16 else mybir.dt.float32
    nc = bacc.Bacc(
        "TRN2",
        target_bir_lowering=False,
        debug=False,
        enable_asserts=False,
        num_devices=N_CORES,
    )
    xw_in, sg_meta = [], []
    for s, (n_yt, n_xt, ww, sbands, gbands) in enumerate(slot_params):
        s_offs, s_tot = _band_offsets(sbands)
        g_offs, g_tot = _band_offsets(gbands)
        sg_meta.append((s_offs, g_offs, s_tot, g_tot))
        xw_in.append(nc.dram_tensor(f"xw{s}", [128, C, n_yt, ww], dtd,
                                    kind="ExternalInput"))
    out = nc.dram_tensor("out", [B_LOC, 112, 2, C, OUT_W], dtd,
                         kind="ExternalOutput")

    n_slots = len(slot_params)
    # head waits on slot_order[0]'s DMA (keep it small-ish); the tail drain
    # is slot_order[-1]'s pass2 (make it the smallest slot)
    slot_order = list(range(1, n_slots)) + [0]
    # S|G bands ship as two blobs: the first two compute slots' pieces (so
    # early matmuls aren't gated on the whole thing), then the rest
    sg_w = [sg_meta[s][2] + sg_meta[s][3] for s in range(n_slots)]
    sg_base = {}
    off0 = 0
    for s in slot_order[:2]:
        sg_base[s] = off0
        off0 += sg_w[s]
    off = 0
    for s in slot_order[2:]:
        sg_base[s] = off
        off += sg_w[s]
    sg0_in = nc.dram_tensor("sg_first", [128, off0], dtd,
                            kind="ExternalInput")
    sgr_in = nc.dram_tensor("sg_rest", [128, off], dtd,
                            kind="ExternalInput")
    with tile.TileContext(nc) as tc:
        with (
            tc.tile_pool(name="img", bufs=n_slots) as img_pool,
            tc.tile_pool(name="sg", bufs=2) as sg_pool,
            tc.tile_pool(name="tmp", bufs=3) as tmp_pool,
            tc.tile_pool(name="outp", bufs=3) as out_pool,
            tc.tile_pool(name="ps1a", bufs=2, space="PSUM") as ps1a_pool,
            tc.tile_pool(name="ps1b", bufs=2, space="PSUM") as ps1b_pool,
            tc.tile_pool(name="ps2a", bufs=2, space="PSUM") as ps2a_pool,
            tc.tile_pool(name="ps2b", bufs=2, space="PSUM") as ps2b_pool,
        ):
            tmp_tiles = [None] * n_slots

            # prefetch everything up front: first slot's S|G, its image,
            # then the remaining S|G blob and the other images
            img_tiles = [None] * n_slots
            first2 = slot_order[:2]
            sg0_sb = sg_pool.tile([128, off0], dtd, tag="sg0")
            nc.sync.dma_start(sg0_sb[:], sg0_in[:])
            for s in first2:
                n_yt, n_xt, ww, _, _ = slot_params[s]
                img_sb = img_pool.tile([128, C, n_yt, ww], dtd)
                nc.sync.dma_start(img_sb[:], xw_in[s][:])
                img_tiles[s] = img_sb
            sgr_sb = sg_pool.tile([128, off], dtd, tag="sgr")
            nc.sync.dma_start(sgr_sb[:], sgr_in[:])
            for s in slot_order[2:]:
                n_yt, n_xt, ww, _, _ = slot_params[s]
                img_sb = img_pool.tile([128, C, n_yt, ww], dtd)
                nc.sync.dma_start(img_sb[:], xw_in[s][:])
                img_tiles[s] = img_sb
            sg_tiles = [(sg0_sb if s in first2 else sgr_sb) for s in
                        range(n_slots)]

            def pass1(s):
                n_yt, n_xt, ww, sbands, gbands = slot_params[s]
                s_offs, g_offs, s_tot, g_tot = sg_meta[s]
                sg_sb = sg_tiles[s]
                img_sb = img_tiles[s]
                s_emit = [t for t in range(n_yt) if sbands[t] is not None]
                tmp_sb = tmp_pool.tile([128, n_xt, C, OUT_H], dtd)
                tmp_tiles[s] = tmp_sb
                for xb in range(n_xt):
                    xlo = xb * 128
                    xn = min(128, ww - xlo)
                    ps1a = ps1a_pool.tile([128, 2 * OUT_H], dt, tag='ps1a')
                    ps1b = ps1b_pool.tile([128, OUT_H], dt, tag='ps1b')
                    # c0+c1 share one accumulation session in ps1a: PSUM
                    # has_written bits make the split ranges exact
                    for c in range(C):
                        base = c * OUT_H if c < 2 else 0
                        ps = ps1a if c < 2 else ps1b
                        for i_t, t in enumerate(s_emit):
                            lo, hi = sbands[t]
                            o = sg_base[s] + s_offs[t]
                            first = i_t == 0 and (c == 0 or c == 2)
                            last = (i_t == len(s_emit) - 1
                                    and (c == 1 or c == 2))
                            nc.tensor.matmul(
                                ps[:xn, base + lo:base + hi],
                                img_sb[:, c, t, xlo:xlo + xn],
                                sg_sb[:, o:o + hi - lo],
                                start=first,
                                stop=last,
                                skip_group_check=True,
                            )
                    # ping-pong the drains so each engine sees half the
                    # burst rate (small slots outrun a single drain engine)
                    if xb % 2 == 0:
                        nc.scalar.copy(tmp_sb[:xn, xb, 0:2, :],
                                       ps1a[:xn, :])
                        nc.vector.tensor_copy(tmp_sb[:xn, xb, 2, :],
                                              ps1b[:xn, :OUT_H])
                    else:
                        nc.vector.tensor_copy(tmp_sb[:xn, xb, 0:2, :],
                                              ps1a[:xn, :])
                        nc.scalar.copy(tmp_sb[:xn, xb, 2, :],
                                       ps1b[:xn, :OUT_H])

            def pass2(s):
                n_yt, n_xt, ww, sbands, gbands = slot_params[s]
                s_offs, g_offs, s_tot, g_tot = sg_meta[s]
                sg_sb = sg_tiles[s]
                tmp_sb = tmp_tiles[s]
                g_emit = [t for t in range(n_xt) if gbands[t] is not None]
                out_sb = out_pool.tile([112, 2, C, OUT_W], dtd, tag="out")
                for jb in range(2):
                    ps2a = ps2a_pool.tile([112, 2 * OUT_W], dt, tag='ps2a')
                    ps2b = ps2b_pool.tile([112, OUT_W], dt, tag='ps2b')
                    for c in range(C):
                        base = c * OUT_W if c < 2 else 0
                        ps = ps2a if c < 2 else ps2b
                        for i_t, xb in enumerate(g_emit):
                            lo, hi = gbands[xb]
                            o = sg_base[s] + s_tot + g_offs[xb]
                            xn = min(128, ww - xb * 128)
                            first = i_t == 0 and (c == 0 or c == 2)
                            last = (i_t == len(g_emit) - 1
                                    and (c == 1 or c == 2))
                            nc.tensor.matmul(
                                ps[:, base + lo:base + hi],
                                tmp_sb[:xn, xb, c,
                                       jb * 112:(jb + 1) * 112],
                                sg_sb[:xn, o:o + hi - lo],
                                start=first,
                                stop=last,
                                skip_group_check=True,
                            )
                    if jb == 0:
                        nc.vector.tensor_copy(out_sb[:, jb, 0:2, :],
                                              ps2a[:, :])
                        nc.scalar.copy(out_sb[:, jb, 2, :],
                                       ps2b[:, :OUT_W])
                    else:
                        nc.scalar.copy(out_sb[:, jb, 0:2, :],
                                       ps2a[:, :])
                        nc.vector.tensor_copy(out_sb[:, jb, 2, :],
                                              ps2b[:, :OUT_W])
                nc.sync.dma_start(out[s][:], out_sb[:])

            # software pipeline: pass2 of the previous slot is emitted after
            # pass1 of the current one, so PE never head-of-line blocks on
            # the ps1->tmp drains.
            prev = None
            for s in slot_order:
                pass1(s)
                if prev is not None:
                    pass2(prev)
                prev = s
            pass2(prev)
    nc.compile()
    return nc


def kernel(x, _trace=False):
    global LAST_EXEC_NS, LAST_RESULTS
    from concourse.bass_utils import run_bass_kernel_spmd

    x = np.ascontiguousarray(np.asarray(x), dtype=np.float32)
    assert x.shape == (B_FULL, C, H, W + 1), x.shape

    slot_params, in_maps, assign = _prepare(x)
    key = (slot_params, USE_F16)
    if key not in _NC_CACHE:
        _NC_CACHE[key] = _build_nc(slot_params)
    nc = _NC_CACHE[key]

    res = run_bass_kernel_spmd(nc, in_maps, list(range(N_CORES)), trace=_trace)
    LAST_EXEC_NS = res.exec_time_ns
    LAST_RESULTS = res

    out_full = np.empty((B_FULL, C, OUT_H, OUT_W), np.float32)
    for s in range(B_LOC):
        for c in range(N_CORES):
            # device layout [112, 2, C, OUT_W]; j = b*112 + p
            arr = res.results[c]["out"][s].astype(np.float32)
            out_full[assign[s][c]] = arr.transpose(2, 1, 0, 3).reshape(
                C, OUT_H, OUT_W)
    return out_full


# revision 79
# speedup vs baseline: 1.1276x; 1.1097x over previous
"""Trainium2 Bass kernel for nn_CenterCrop: per-sample resize(short-side=256)
+ center-crop(224), bilinear, batch sharded over 8 NeuronCores.

Bilinear resize is separable: out = S^T @ img @ G with per-sample sparse
interpolation matrices S (vertical) and G (horizontal), built on the host
from the h/w metadata. The gather+lerp runs on the PE array as matmuls:
  pass1: tmp1_T[x, j] = sum_y img[y, x] * S[y, j]   (img tiles stationary)
  pass2: out[j, i]    = sum_x tmp1_T[x, j] * G[x, i] (tmp1 tiles stationary)

Perf structure (baseline fp32 115.5us -> ~44us measured):
- fp16 data path end-to-end: PE streams at 1 cyc/row (4x the fp32 rate),
  DMA bytes halve, output written fp16 and upcast on host. Bilinear error
  stays ~8e-4 max-rel (gate is 2e-2).
- Only the per-sample source window that the output reads is DMA'd, in a
  partition-major layout so every DMA is one linear run per partition.
- S/G are banded (2 nonzeros per column); only the per-128-row-tile
  nonzero band columns ship, packed into two blobs (first two compute
  slots' bands alone so the first matmul isn't gated on the rest).
- All input DMAs are issued up front (img bufs = n_slots, no reuse WAR).
- PSUM channel pairing: c0+c1 share a [*,448] PSUM tile and a single
  accumulation session (has_written bits make split ranges exact), so
  each x-chunk drains in 2 wide casts instead of 3 narrow ones; the two
  drain engines (Act, DVE) ping-pong per x-chunk/jb so neither sees the
  full burst rate (GpSimd cannot touch PSUM, DMA cannot read it).
- Software pipelining: pass2 of slot s-1 is emitted after pass1 of slot
  s, so the PE queue never head-of-line blocks on the ps1->tmp drains.
- SPMD requires one program for all 8 cores, so samples are sorted by
  min(h,w) and dealt round-robin: slot s on every core holds same-sized
  windows; the program is specialized per-slot to the union shape/bands.
  Outputs are unpermuted/transposed back on the host.
"""

import sys
import os

for _p in ("/opt/trn_rl_repo",):
    if os.path.isdir(_p) and _p not in sys.path:
        sys.path.insert(0, _p)

import numpy as np

OUT_H = 224
OUT_W = 224
RESIZE_TO = np.float32(256.0)
B_FULL = 64
N_CORES = 8
B_LOC = B_FULL // N_CORES  # 8 slots per core
C = 3
H = 512
W = 512  # image width after stripping the metadata column (stored width 513)

LAST_EXEC_NS = None
LAST_RESULTS = None
_NC_CACHE = {}

# fp16 single-pass data path (default). Disable via CENTERCROP_F16=0 for an
# fp32 debugging fallback.
USE_F16 = os.environ.get("CENTERCROP_F16", "1") == "1"


def _interp_matrices(h, w):
    """Full S [512, OUT_H], G [512, OUT_W] fp32 interpolation matrices,
    mirroring the reference fp32 math bit-for-bit."""
    f32 = np.float32
    h = f32(h)
    w = f32(w)
    min_dim = min(h, w)
    scale = RESIZE_TO / min_dim
    h_res = np.round(h * scale)
    w_res = np.round(w * scale)
    top = np.round((h_res - f32(OUT_H)) / f32(2.0))
    left = np.round((w_res - f32(OUT_W)) / f32(2.0))

    def axis_mat(n_out, offset, dim, dim_res, n_src):
        idx = np.arange(n_out, dtype=np.float32) + offset
        src = np.clip((idx + f32(0.5)) * dim / dim_res - f32(0.5),
                      f32(0.0), dim - f32(1.0))
        p0f = np.floor(src)
        frac = src - p0f
        imax = np.int32(dim) - 1
        p0 = np.clip(p0f.astype(np.int32), 0, imax)
        p1 = np.minimum(p0 + 1, imax)
        mat = np.zeros((n_src, n_out), np.float32)
        cols = np.arange(n_out)
        np.add.at(mat, (p0, cols), f32(1.0) - frac)
        np.add.at(mat, (p1, cols), frac)
        return mat

    S = axis_mat(OUT_H, top, h, h_res, H)
    G = axis_mat(OUT_W, left, w, w_res, W)
    return S, G


def _bands(mat_w, n_tiles):
    """Per-128-row-tile [lo, hi) columns with any nonzero; None if empty."""
    out = []
    for t in range(n_tiles):
        rows = mat_w[t * 128:(t + 1) * 128]
        nz = np.nonzero(rows.any(axis=0))[0]
        out.append(None if nz.size == 0 else (int(nz[0]), int(nz[-1]) + 1))
    return out


def _union_bands(band_lists):
    n = len(band_lists[0])
    out = []
    for t in range(n):
        los = [b[t][0] for b in band_lists if b[t] is not None]
        his = [b[t][1] for b in band_lists if b[t] is not None]
        out.append(None if not los else (min(los), max(his)))
    return out


def _band_offsets(bands):
    """Packed running offsets for non-empty bands; total width last."""
    offs = []
    off = 0
    for b in bands:
        if b is None:
            offs.append(None)
        else:
            offs.append(off)
            off += b[1] - b[0]
    return offs, off


def _prepare(x):
    """Host prep: per-sample windows/matrices, sorted slot assignment,
    per-core packed inputs, and the per-slot program parameters."""
    dtd_np = np.float16 if USE_F16 else np.float32
    h_all = x[:, 0, 0, -1].astype(np.float32)
    w_all = x[:, 1, 0, -1].astype(np.float32)

    samples = []
    for b in range(B_FULL):
        S, G = _interp_matrices(h_all[b], w_all[b])
        ynz = np.nonzero(S.any(axis=1))[0]
        xnz = np.nonzero(G.any(axis=1))[0]
        y0, y1 = int(ynz[0]), int(ynz[-1]) + 1
        x0, x1 = int(xnz[0]), int(xnz[-1]) + 1
        samples.append(dict(S=S[y0:y1], G=G[x0:x1], y0=y0, x0=x0,
                            wh=y1 - y0, ww=x1 - x0))

    order = np.argsort(np.minimum(h_all, w_all), kind="stable")
    # slot s, core c -> sample order[s*N_CORES + c]
    assign = [[int(order[s * N_CORES + c]) for c in range(N_CORES)]
              for s in range(B_LOC)]

    slot_params = []
    slot_data = []  # per slot: list over cores of (sid, Sw_pad, Gw_pad)
    for s in range(B_LOC):
        sids = assign[s]
        wh = max(samples[i]["wh"] for i in sids)
        ww = max(samples[i]["ww"] for i in sids)
        n_yt = (wh + 127) // 128
        n_xt = (ww + 127) // 128
        sb_list, gb_list, data = [], [], []
        for i in sids:
            sp = samples[i]
            Sw = np.zeros((n_yt * 128, OUT_H), np.float32)
            Sw[:sp["wh"]] = sp["S"]
            Gw = np.zeros((n_xt * 128, OUT_W), np.float32)
            Gw[:sp["ww"]] = sp["G"]
            sb_list.append(_bands(Sw, n_yt))
            gb_list.append(_bands(Gw, n_xt))
            data.append((i, Sw, Gw))
        sbands = _union_bands(sb_list)
        gbands = _union_bands(gb_list)
        slot_params.append((n_yt, n_xt, ww,
                            tuple(sbands), tuple(gbands)))
        slot_data.append(data)

    # pack per-core input maps; all slots' S|G bands share one "sgall"
    in_maps = [{} for _ in range(N_CORES)]
    sg_parts = [[] for _ in range(N_CORES)]
    for s in range(B_LOC):
        n_yt, n_xt, ww, sbands, gbands = slot_params[s]
        s_offs, s_tot = _band_offsets(sbands)
        g_offs, g_tot = _band_offsets(gbands)
        for c in range(N_CORES):
            sid, Sw, Gw = slot_data[s][c]
            sp = samples[sid]
            xw = np.zeros((C, n_yt, 128, ww), dtd_np)
            win = x[sid, :, sp["y0"]:sp["y0"] + sp["wh"],
                    sp["x0"]:sp["x0"] + sp["ww"]]
            for t in range(n_yt):
                rows = win[:, t * 128:(t + 1) * 128]
                xw[:, t, :rows.shape[1], :sp["ww"]] = rows
            # partition-major layout so the DMA is one linear run/partition
            xw = np.ascontiguousarray(xw.transpose(2, 0, 1, 3))
            # packed S|G bands: [128, s_tot + g_tot]
            sg = np.zeros((128, s_tot + g_tot), dtd_np)
            St = Sw.reshape(n_yt, 128, OUT_H)
            Gt = Gw.reshape(n_xt, 128, OUT_W)
            for t in range(n_yt):
                if sbands[t] is not None:
                    lo, hi = sbands[t]
                    sg[:, s_offs[t]:s_offs[t] + hi - lo] = St[t, :, lo:hi]
            for t in range(n_xt):
                if gbands[t] is not None:
                    lo, hi = gbands[t]
                    o = s_tot + g_offs[t]
                    sg[:, o:o + hi - lo] = Gt[t, :, lo:hi]
            in_maps[c][f"xw{s}"] = xw
            sg_parts[c].append(sg)
    order = list(range(1, B_LOC)) + [0]
    for c in range(N_CORES):
        in_maps[c]["sg_first"] = np.concatenate(
            [sg_parts[c][s] for s in order[:2]], axis=1)
        in_maps[c]["sg_rest"] = np.concatenate(
            [sg_parts[c][s] for s in order[2:]], axis=1)
    return tuple(slot_params), in_maps, assign


def _build_nc(slot_params):
    import concourse.bacc as bacc
    import concourse.mybir as mybir
    import concourse.tile as tile

    dt = mybir.dt.float32
    dtd = mybir.dt.float16 if USE_F16 else mybir.dt.float32
    nc = bacc.Bacc(
        "TRN2",
        target_bir_lowering=False,
        debug=False,
        enable_asserts=False,
        num_devices=N_CORES,
    )
    xw_in, sg_meta = [], []
    for s, (n_yt, n_xt, ww, sbands, gbands) in enumerate(slot_params):
        s_offs, s_tot = _band_offsets(sbands)
        g_offs, g_tot = _band_offsets(gbands)
        sg_meta.append((s_offs, g_offs, s_tot, g_tot))
        xw_in.append(nc.dram_tensor(f"xw{s}", [128, C, n_yt, ww], dtd,
                                    kind="ExternalInput"))
    out = nc.dram_tensor("out", [B_LOC, 112, 2, C, OUT_W], dtd,
                         kind="ExternalOutput")

    n_slots = len(slot_params)
    # head waits on slot_order[0]'s DMA (keep it small-ish); the tail drain
    # is slot_order[-1]'s pass2 (make it the smallest slot)
    slot_order = list(range(1, n_slots)) + [0]
    # S|G bands ship as two blobs: the first two compute slots' pieces (so
    # early matmuls aren't gated on the whole thing), then the rest
    sg_w = [sg_meta[s][2] + sg_meta[s][3] for s in range(n_slots)]
    sg_base = {}
    off0 = 0
    for s in slot_order[:2]:
        sg_base[s] = off0
        off0 += sg_w[s]
    off = 0
    for s in slot_order[2:]:
        sg_base[s] = off
        off += sg_w[s]
    sg0_in = nc.dram_tensor("sg_first", [128, off0], dtd,
                            kind="ExternalInput")
    sgr_in = nc.dram_tensor("sg_rest", [128, off], dtd,
                            kind="ExternalInput")
    with tile.TileContext(nc) as tc:
        with (
            tc.tile_pool(name="img", bufs=n_slots) as img_pool,
            tc.tile_pool(name="sg", bufs=2) as sg_pool,
            tc.tile_pool(name="tmp", bufs=3) as tmp_pool,
            tc.tile_pool(name="outp", bufs=3) as out_pool,
            tc.tile_pool(name="ps1a", bufs=3, space="PSUM") as ps1a_pool,
            tc.tile_pool(name="ps1b", bufs=2, space="PSUM") as ps1b_pool,
            tc.tile_pool(name="ps2", bufs=3, space="PSUM") as ps2_pool,
        ):
            tmp_tiles = [None] * n_slots

            # prefetch everything up front: first slot's S|G, its image,
            # then the remaining S|G blob and the other images
            img_tiles = [None] * n_slots
            first2 = slot_order[:2]
            sg0_sb = sg_pool.tile([128, off0], dtd, tag="sg0")
            nc.sync.dma_start(sg0_sb[:], sg0_in[:])
            for s in first2:
                n_yt, n_xt, ww, _, _ = slot_params[s]
                img_sb = img_pool.tile([128, C, n_yt, ww], dtd)
                nc.sync.dma_start(img_sb[:], xw_in[s][:])
                img_tiles[s] = img_sb
            sgr_sb = sg_pool.tile([128, off], dtd, tag="sgr")
            nc.sync.dma_start(sgr_sb[:], sgr_in[:])
            for s in slot_order[2:]:
                n_yt, n_xt, ww, _, _ = slot_params[s]
                img_sb = img_pool.tile([128, C, n_yt, ww], dtd)
                nc.sync.dma_start(img_sb[:], xw_in[s][:])
                img_tiles[s] = img_sb
            sg_tiles = [(sg0_sb if s in first2 else sgr_sb) for s in
                        range(n_slots)]

            def pass1(s):
                n_yt, n_xt, ww, sbands, gbands = slot_params[s]
                s_offs, g_offs, s_tot, g_tot = sg_meta[s]
                sg_sb = sg_tiles[s]
                img_sb = img_tiles[s]
                s_emit = [t for t in range(n_yt) if sbands[t] is not None]
                tmp_sb = tmp_pool.tile([128, n_xt, C, OUT_H], dtd)
                tmp_tiles[s] = tmp_sb
                for xb in range(n_xt):
                    xlo = xb * 128
                    xn = min(128, ww - xlo)
                    ps1a = ps1a_pool.tile([128, 2 * OUT_H], dt, tag='ps1a')
                    ps1b = ps1b_pool.tile([128, OUT_H], dt, tag='ps1b')
                    # c0+c1 share one accumulation session in ps1a: PSUM
                    # has_written bits make the split ranges exact
                    for c in range(C):
                        base = c * OUT_H if c < 2 else 0
                        ps = ps1a if c < 2 else ps1b
                        for i_t, t in enumerate(s_emit):
                            lo, hi = sbands[t]
                            o = sg_base[s] + s_offs[t]
                            first = i_t == 0 and (c == 0 or c == 2)
                            last = (i_t == len(s_emit) - 1
                                    and (c == 1 or c == 2))
                            nc.tensor.matmul(
                                ps[:xn, base + lo:base + hi],
                                img_sb[:, c, t, xlo:xlo + xn],
                                sg_sb[:, o:o + hi - lo],
                                start=first,
                                stop=last,
                                skip_group_check=True,
                            )
                    # ping-pong the drains so each engine sees half the
                    # burst rate (small slots outrun a single drain engine)
                    if xb % 2 == 0:
                        nc.scalar.copy(tmp_sb[:xn, xb, 0:2, :],
                                       ps1a[:xn, :])
                        nc.vector.tensor_copy(tmp_sb[:xn, xb, 2, :],
                                              ps1b[:xn, :OUT_H])
                    else:
                        nc.vector.tensor_copy(tmp_sb[:xn, xb, 0:2, :],
                                              ps1a[:xn, :])
                        nc.scalar.copy(tmp_sb[:xn, xb, 2, :],
                                       ps1b[:xn, :OUT_H])

            def pass2(s):
                n_yt, n_xt, ww, sbands, gbands = slot_params[s]
                s_offs, g_offs, s_tot, g_tot = sg_meta[s]
                sg_sb = sg_tiles[s]
                tmp_sb = tmp_tiles[s]
                g_emit = [t for t in range(n_xt) if gbands[t] is not None]
                out_sb = out_pool.tile([112, 2, C, OUT_W], dtd, tag="out")
                for jb in range(2):
                    ps2a = ps2_pool.tile([112, 2 * OUT_W], dt, tag='ps2')
                    ps2b = ps2_pool.tile([112, OUT_W], dt, tag='ps2')
                    for c in range(C):
                        base = c * OUT_W if c < 2 else 0
                        ps = ps2a if c < 2 else ps2b
                        for i_t, xb in enumerate(g_emit):
                            lo, hi = gbands[xb]
                            o = sg_base[s] + s_tot + g_offs[xb]
                            xn = min(128, ww - xb * 128)
                            first = i_t == 0 and (c == 0 or c == 2)
                            last = (i_t == len(g_emit) - 1
                                    and (c == 1 or c == 2))
                            nc.tensor.matmul(
                                ps[:, base + lo:base + hi],
                                tmp_sb[:xn, xb, c,
                                       jb * 112:(jb + 1) * 112],
                                sg_sb[:xn, o:o + hi - lo],
                                start=first,
                                stop=last,
                                skip_group_check=True,
                            )
                    if jb == 0:
                        nc.vector.tensor_copy(out_sb[:, jb, 0:2, :],
                                              ps2a[:, :])
                        nc.scalar.copy(out_sb[:, jb, 2, :],
                                       ps2b[:, :OUT_W])
                    else:
                        nc.scalar.copy(out_sb[:, jb, 0:2, :],
                                       ps2a[:, :])
                        nc.vector.tensor_copy(out_sb[:, jb, 2, :],
                                              ps2b[:, :OUT_W])
                nc.sync.dma_start(out[s][:], out_sb[:])

            # software pipeline: pass2 of the previous slot is emitted after
            # pass1 of the current one, so PE never head-of-line blocks on
            # the ps1->tmp drains.
            prev = None
            for s in slot_order:
                pass1(s)
                if prev is not None:
                    pass2(prev)
                prev = s
            pass2(prev)
    nc.compile()
    return nc


def kernel(x, _trace=False):
    global LAST_EXEC_NS, LAST_RESULTS
    from concourse.bass_utils import run_bass_kernel_spmd

    x = np.ascontiguousarray(np.asarray(x), dtype=np.float32)
    assert x.shape == (B_FULL, C, H, W + 1), x.shape

    slot_params, in_maps, assign = _prepare(x)
    key = (slot_params, USE_F16)
    if key not in _NC_CACHE:
        _NC_CACHE[key] = _build_nc(slot_params)
    nc = _NC_CACHE[key]

    res = run_bass_kernel_spmd(nc, in_maps, list(range(N_CORES)), trace=_trace)
    LAST_EXEC_NS = res.exec_time_ns
    LAST_RESULTS = res

    out_full = np.empty((B_FULL, C, OUT_H, OUT_W), np.float32)
    for s in range(B_LOC):
        for c in range(N_CORES):
            # device layout [112, 2, C, OUT_W]; j = b*112 + p
            arr = res.results[c]["out"][s].astype(np.float32)
            out_full[assign[s][c]] = arr.transpose(2, 1, 0, 3).reshape(
                C, OUT_H, OUT_W)
    return out_full
